# revision 1
# baseline (speedup 1.0000x reference)
"""Attention-pooling kernel for TRN2 (8 NeuronCores, batch-parallel).

Computes, for x:[32,2048,1024], W:[1024,1024], b:[1024], ctx:[1024]:
    h = tanh(x @ W + b); scores = h . ctx
    weights = softmax(scores, axis=seq)
    out = sum_s weights[s] * x[s]          -> [32, 1024]

Sharding: data-parallel over batch, 4 batches per core.

Per core: all matmuls run in float32r (full PE rate at N>=512, ~13-bit
mantissa; inputs are DMA'd with a bitcast — the PE truncates f32r
operands on read, verified bit-identical to explicit rounding).
Pass 1 computes h^T = W^T x^T per 512-row seq tile: PE transposes of x
(interleaved between matmul groups to keep the HAM clock-gate warm),
f32r matmuls accumulating h^T in PSUM, tanh+bias on ScalarE, and the
ctx-dot as an M=1 f32r matmul back on the PE producing scores [1, S].
Softmax is unnormalized (exp(s - max); the 1/Z lands on the pooled
vector). Pass 2 pools x with the transposed weight vector as the
stationary operand; it is emitted one tile late so the PE's in-order
queue never stalls on the softmax chain.
"""

import numpy as np
from contextlib import ExitStack

import concourse.bacc as bacc
import concourse.mybir as mybir
import concourse.tile as tile
from concourse import masks
from concourse.bass_utils import run_bass_kernel_spmd

B, S, E, A = 32, 2048, 1024, 1024
NCORES = 8
BL = B // NCORES          # batches per core
S_TILE = 512
NCH = S_TILE // 128       # S chunks (128 rows) per tile
NT = S // S_TILE          # seq tiles per batch
KE = E // 128             # contraction chunks over embed dim
KA = A // 128             # chunks over attention dim
NC2 = S // 128            # S chunks per batch (pass 2)

F32 = mybir.dt.float32
F32R = mybir.dt.float32r
AX = mybir.AxisListType.X
AF = mybir.ActivationFunctionType


def _build(reps=1):
    nc = bacc.Bacc("TRN2", target_bir_lowering=False, debug=False,
                   num_devices=NCORES)
    x_d = nc.declare_dram_parameter("x", [BL * S, E], F32, isOutput=False)
    W_d = nc.declare_dram_parameter("W", [E, A], F32, isOutput=False)
    b_d = nc.declare_dram_parameter("b", [A], F32, isOutput=False)
    c_d = nc.declare_dram_parameter("ctx", [A], F32, isOutput=False)
    o_d = nc.declare_dram_parameter("out", [BL, E], F32, isOutput=True)

    with ExitStack() as ctx:
        tc = ctx.enter_context(tile.TileContext(nc))

        const_pool = ctx.enter_context(tc.tile_pool(name="const", bufs=1))
        xr_pool = ctx.enter_context(tc.tile_pool(name="xr", bufs=6))
        xt_pool = ctx.enter_context(tc.tile_pool(name="xT", bufs=2))
        h_pool = ctx.enter_context(tc.tile_pool(name="h", bufs=4))
        sc_pool = ctx.enter_context(tc.tile_pool(name="scores", bufs=1))
        sm_pool = ctx.enter_context(tc.tile_pool(name="softmax", bufs=1))
        out_pool = ctx.enter_context(tc.tile_pool(name="outs", bufs=1))

        ps_t = ctx.enter_context(tc.tile_pool(name="ps_t", bufs=3, space="PSUM"))
        ps_h = ctx.enter_context(tc.tile_pool(name="ps_h", bufs=2, space="PSUM"))
        ps_s = ctx.enter_context(tc.tile_pool(name="ps_s", bufs=1, space="PSUM"))
        ps_o = ctx.enter_context(tc.tile_pool(name="ps_o", bufs=1, space="PSUM"))

        # ---- constants ----
        ident = const_pool.tile([128, 128], F32)
        masks.make_identity(nc, ident[:])
        ident_r = const_pool.tile([128, 128], F32R)
        nc.vector.tensor_copy(ident_r[:], ident[:])
        neg_ones = const_pool.tile([1, 128], F32)
        nc.gpsimd.memset(neg_ones[:], -1.0)

        W_r = const_pool.tile([128, KE * A], F32R)
        b_sb = const_pool.tile([128, KA], F32)
        ctx_r = const_pool.tile([128, KA], F32R)

        tiles = [(rep, bi, t)
                 for rep in range(reps) for bi in range(BL) for t in range(NT)]

        def dma_tile(bi, t):
            r0 = bi * S + t * S_TILE
            xr = xr_pool.tile([128, NCH * E], F32R, tag="xr")
            # per-chunk DMAs: the first transposes gate on 512KB, not 2MB
            for c in range(NCH):
                nc.sync.dma_start(
                    xr[:, c * E:(c + 1) * E],
                    x_d[r0 + c * 128: r0 + (c + 1) * 128, :].bitcast(F32R))
            return xr

        def transpose_group(xr_src, xT_dst, k):
            # 4 transposes: chunk c of S rows, contraction chunk k
            for c in range(NCH):
                tp = ps_t.tile([128, 128], F32R, tag="tps")
                nc.tensor.transpose(
                    tp[:], xr_src[:, c * E + k * 128: c * E + (k + 1) * 128],
                    ident_r[:])
                dst = xT_dst[:, k * S_TILE + c * 128: k * S_TILE + (c + 1) * 128]
                if c % 2 == 0:
                    nc.scalar.activation(dst, tp[:], AF.Copy)
                else:
                    nc.vector.tensor_copy(dst, tp[:])

        def flush_pass2(pend):
            scores_sb, pmax_sb, batch_xrs, orow = pend
            # Transpose the RAW scores first (no softmax dependency), then
            # exponentiate in the [128, NC2] layout: the transposes never
            # wait on the softmax chain and the exp is 128-lane-parallel.
            sT = sm_pool.tile([128, NC2], F32, tag="sT")
            for g in range(NC2 // 8):
                tp = ps_t.tile([128, 8], F32, tag="tps")
                for u in range(8):
                    c2 = g * 8 + u
                    nc.tensor.matmul(
                        tp[:, u:u + 1], scores_sb[0:1, c2 * 128:(c2 + 1) * 128],
                        ident[0:1, 0:1],
                        is_transpose=True,
                        start=(u == 0), stop=(u == 7),
                        skip_group_check=True)
                nc.scalar.activation(sT[:, g * 8:(g + 1) * 8], tp[:], AF.Copy)

            # global max from the per-tile partials; broadcast -max to all
            # partitions with a K=1 matmul against a -1s row
            m_sb = sm_pool.tile([1, 1], F32, tag="m")
            nc.vector.reduce_max(m_sb[:], pmax_sb[:], axis=AX)
            mb_ps = ps_t.tile([128, 1], F32, tag="tps")
            nc.tensor.matmul(mb_ps[:], neg_ones[:], m_sb[:],
                             start=True, stop=True)
            mb = sm_pool.tile([128, 1], F32, tag="mb")
            nc.scalar.activation(mb[:], mb_ps[:], AF.Copy)

            # exp -> pass-2 stationary operand; accum gives per-partition Z
            pT = sm_pool.tile([128, NC2], F32R, tag="pT")
            zc = sm_pool.tile([128, 1], F32, tag="zc")
            nc.scalar.activation(pT[:], sT[:], AF.Exp, bias=mb[:, 0:1],
                                 accum_out=zc[:])
            # cross-partition sum of zc -> Z -> 1/Z
            zt = ps_t.tile([1, 128], F32, tag="tps")
            nc.tensor.transpose(zt[:], zc[:], ident[:])
            zrow = sm_pool.tile([1, 128], F32, tag="zrow")
            nc.scalar.activation(zrow[:], zt[:], AF.Copy)
            z_sb = sm_pool.tile([1, 1], F32, tag="z")
            nc.vector.reduce_sum(z_sb[:], zrow[:], axis=AX)
            rz = sm_pool.tile([1, 1], F32, tag="rz")
            nc.vector.reciprocal(rz[:], z_sb[:])

            # pass 2: pooling
            op0 = ps_o.tile([1, 512], F32, tag="op0")
            op1 = ps_o.tile([1, 512], F32, tag="op1")
            for c2 in range(NC2):
                xsrc = batch_xrs[c2 // NCH]
                cc = c2 % NCH
                nc.tensor.matmul(op0[:], pT[:, c2:c2 + 1],
                                 xsrc[:, cc * E: cc * E + 512],
                                 start=(c2 == 0), stop=(c2 == NC2 - 1))
                nc.tensor.matmul(op1[:], pT[:, c2:c2 + 1],
                                 xsrc[:, cc * E + 512: (cc + 1) * E],
                                 start=(c2 == 0), stop=(c2 == NC2 - 1))

            ob = out_pool.tile([1, E], F32, tag="ob")
            nc.vector.tensor_scalar_mul(ob[:, 0:512], op0[:], rz[0:1, 0:1])
            nc.vector.tensor_scalar_mul(ob[:, 512:1024], op1[:], rz[0:1, 0:1])
            nc.sync.dma_start(o_d[orow:orow + 1, :], ob[:])

        # prologue ordering: first x tile's DMA goes out before the 4MB W
        # load so the transposes can start ASAP; W chunk k only gates the
        # k-th matmul of the first accumulation group.
        xr_cur = dma_tile(tiles[0][1], tiles[0][2])
        # W -> [128, KE*A] f32r (col k*A+a holds W[k*128+p, a]), per chunk
        for k in range(KE):
            nc.sync.dma_start(
                W_r[:, k * A:(k + 1) * A],
                W_d[k * 128:(k + 1) * 128, :].bitcast(F32R))
        nc.sync.dma_start(b_sb[:], b_d.rearrange("(j p) -> p j", p=128))
        nc.sync.dma_start(ctx_r[:],
                          c_d.bitcast(F32R).rearrange("(j p) -> p j", p=128))

        # warm the PE HAM clock-gate with throwaway matmuls while the first
        # DMAs land (the PE would otherwise idle cold and re-throttle)
        warm_scratch = out_pool.tile([128, 512], F32, tag="ob",
                                     name="warm_scratch")
        for w in range(28):
            wp = ps_h.tile([128, 128], F32, tag="hps", name=f"warm{w}")
            nc.tensor.matmul(wp[:], ident[:], ident[:], start=True, stop=True)
            if w % 14 == 13:
                nc.scalar.activation(warm_scratch[:, 0:128], wp[:], AF.Copy)

        # first tile's transposes
        xT_cur = xt_pool.tile([128, KE * S_TILE], F32R, tag="xT")
        for k in range(KE):
            transpose_group(xr_cur, xT_cur, k)

        pending = None
        scores_sb = None
        batch_xrs = []

        for i, (rep, bi, t) in enumerate(tiles):
            if t == 0:
                scores_sb = sc_pool.tile([1, S], F32, tag="scores")
                pmax_sb = sc_pool.tile([1, NT], F32, tag="pmax")
                batch_xrs = []
            batch_xrs.append(xr_cur)

            if t == 1 and pending is not None:
                flush_pass2(pending)
                pending = None

            nxt = tiles[i + 1] if i + 1 < len(tiles) else None
            if nxt is not None:
                xr_next = dma_tile(nxt[1], nxt[2])
                xT_next = xt_pool.tile([128, KE * S_TILE], F32R, tag="xT")
            else:
                xr_next = xT_next = None

            sc_ps = ps_s.tile([1, S_TILE], F32, tag="scps")
            for j in range(KA):
                hp = ps_h.tile([128, S_TILE], F32, tag="hps")
                for k in range(KE):
                    nc.tensor.matmul(
                        hp[:],
                        W_r[:, k * A + j * 128: k * A + (j + 1) * 128],
                        xT_cur[:, k * S_TILE:(k + 1) * S_TILE],
                        start=(k == 0), stop=(k == KE - 1))
                h_sb = h_pool.tile([128, S_TILE], F32R, tag="h")
                nc.scalar.activation(h_sb[:], hp[:], AF.Tanh,
                                     bias=b_sb[:, j:j + 1])
                # next tile's transposes ride between matmul groups (keeps
                # the PE warm) and ahead of the ctx-dot, which has to wait
                # for the tanh — the transposes only need the DMA
                if xT_next is not None:
                    transpose_group(xr_next, xT_next, j)
                nc.tensor.matmul(sc_ps[:], ctx_r[:, j:j + 1], h_sb[:],
                                 start=(j == 0), stop=(j == KA - 1))
            nc.vector.tensor_copy(scores_sb[:, t * S_TILE:(t + 1) * S_TILE],
                                  sc_ps[:])
            nc.vector.reduce_max(pmax_sb[:, t:t + 1], sc_ps[:], axis=AX)

            if t == NT - 1:
                pending = (scores_sb, pmax_sb, list(batch_xrs), bi)

            xr_cur, xT_cur = xr_next, xT_next

        if pending is not None:
            flush_pass2(pending)

    nc.compile()
    return nc


_NC_CACHE = None


def kernel(x, W, b, ctx):
    global _NC_CACHE
    if _NC_CACHE is None:
        _NC_CACHE = _build()
    nc = _NC_CACHE

    x = np.ascontiguousarray(np.asarray(x, dtype=np.float32))
    W = np.ascontiguousarray(np.asarray(W, dtype=np.float32))
    b = np.ascontiguousarray(np.asarray(b, dtype=np.float32))
    ctx = np.ascontiguousarray(np.asarray(ctx, dtype=np.float32))

    in_maps = [
        {"x": x[i * BL:(i + 1) * BL].reshape(BL * S, E), "W": W, "b": b,
         "ctx": ctx}
        for i in range(NCORES)
    ]
    res = run_bass_kernel_spmd(nc, in_maps, core_ids=list(range(NCORES)))
    return np.concatenate([res.results[i]["out"] for i in range(NCORES)],
                          axis=0)


if __name__ == "__main__":
    rng = np.random.default_rng(0)
    x = rng.standard_normal((B, S, E), dtype=np.float32)
    W = rng.standard_normal((E, A), dtype=np.float32) / np.sqrt(E)
    b = rng.standard_normal((A,), dtype=np.float32) * 0.01
    c = rng.standard_normal((A,), dtype=np.float32)
    out = kernel(x=x, W=W, b=b, ctx=c)
    print(out.shape, out.dtype)



# revision 5
# speedup vs baseline: 1.1360x; 1.1360x over previous
"""Attention-pooling kernel for TRN2 (8 NeuronCores, batch-parallel).

Computes, for x:[32,2048,1024], W:[1024,1024], b:[1024], ctx:[1024]:
    h = tanh(x @ W + b); scores = h . ctx
    weights = softmax(scores, axis=seq)
    out = sum_s weights[s] * x[s]          -> [32, 1024]

Sharding: data-parallel over batch, 4 batches per core.

x and W are cast to fp16 on the host (10-bit mantissa keeps the score
error close to the f32r baseline) so the kernel can use the xbar DMA
transpose: each 512-row seq sub-tile of x is transposed HBM -> SBUF by
the DMA engines directly into a [128, 512, 8] tile with
tile[p, s, k] = x[s, 128k + p], i.e. each embed-block k is a stride-8
moving operand for the PE. This removes all PE transposes and their
PSUM-evacuation copies from the baseline. Pass 1 is then a pure fp16
matmul stream (full PE rate); tanh+bias on ScalarE; the ctx-dot rides
one accumulation group behind so the PE never waits on the tanh. The
softmax is unnormalized (exp(s - max); 1/Z lands on the pooled
vector). Pass 2 pools a straight fp16 copy of x with the exp'd weight
vector as the stationary operand, one batch behind pass 1.
"""

import numpy as np
from contextlib import ExitStack

import concourse.bacc as bacc
import concourse.mybir as mybir
import concourse.tile as tile
from concourse import masks
from concourse.bass_utils import run_bass_kernel_spmd

B, S, E, A = 32, 2048, 1024, 1024
NCORES = 8
BL = B // NCORES          # batches per core
ST = 512                  # seq sub-tile
NSUB = S // ST            # sub-tiles per batch
KE = E // 128             # contraction chunks over embed dim
KA = A // 128             # chunks over attention dim
NC2 = S // 128            # S chunks per batch (pass 2)

F32 = mybir.dt.float32
F16 = mybir.dt.float16
AX = mybir.AxisListType.X
AF = mybir.ActivationFunctionType


def _build(reps=1):
    nc = bacc.Bacc("TRN2", target_bir_lowering=False, debug=False,
                   num_devices=NCORES)
    x_d = nc.declare_dram_parameter("x", [BL * S, E], F16, isOutput=False)
    W_d = nc.declare_dram_parameter("W", [E, A], F16, isOutput=False)
    b_d = nc.declare_dram_parameter("b", [A], F32, isOutput=False)
    c_d = nc.declare_dram_parameter("ctx", [A], F16, isOutput=False)
    o_d = nc.declare_dram_parameter("out", [BL, E], F32, isOutput=True)

    with ExitStack() as ctx:
        tc = ctx.enter_context(tile.TileContext(nc))

        const_pool = ctx.enter_context(tc.tile_pool(name="const", bufs=1))
        xb_pool = ctx.enter_context(tc.tile_pool(name="xb", bufs=2))
        xt_pool = ctx.enter_context(tc.tile_pool(name="xT", bufs=3))
        h_pool = ctx.enter_context(tc.tile_pool(name="h", bufs=4))
        sc_pool = ctx.enter_context(tc.tile_pool(name="scores", bufs=2))
        sm_pool = ctx.enter_context(tc.tile_pool(name="softmax", bufs=1))
        out_pool = ctx.enter_context(tc.tile_pool(name="outs", bufs=1))

        ps_h = ctx.enter_context(tc.tile_pool(name="ps_h", bufs=2, space="PSUM"))
        ps_s = ctx.enter_context(tc.tile_pool(name="ps_s", bufs=2, space="PSUM"))
        ps_t = ctx.enter_context(tc.tile_pool(name="ps_t", bufs=2, space="PSUM"))
        ps_o = ctx.enter_context(tc.tile_pool(name="ps_o", bufs=1, space="PSUM"))

        # ---- constants ----
        ident = const_pool.tile([128, 128], F32)
        masks.make_identity(nc, ident[:])
        neg_ones = const_pool.tile([1, 128], F32)
        nc.gpsimd.memset(neg_ones[:], -1.0)

        W_sb = const_pool.tile([128, KE * A], F16)
        b_sb = const_pool.tile([128, KA], F32)
        ctx_sb = const_pool.tile([128, KA], F16)

        tiles = [(rep, bi, t)
                 for rep in range(reps) for bi in range(BL) for t in range(NSUB)]

        def dma_xt(bi, t):
            # xbar DMA transpose: x rows [512, 1024] -> [128, 8, 512] with
            # xT[p, k, s] = x[r0 + s, 128k + p] (the k dim is "logically part
            # of the partition dim": out logical column e = 128k + p)
            r0 = bi * S + t * ST
            xT = xt_pool.tile([128, KE, ST], F16, tag="xT")
            nc.sync.dma_start_transpose(xT[:], x_d[r0:r0 + ST, :])
            return xT

        def dma_xb(bi):
            # straight fp16 copy for pass-2 pooling: [128, 16, 1024] with
            # xb[p, c, e] = x[bi*S + 128c + p, e]
            xb = xb_pool.tile([128, NC2, E], F16, tag="xb")
            nc.sync.dma_start(
                xb[:], x_d[bi * S:(bi + 1) * S, :].rearrange(
                    "(c p) e -> p c e", p=128))
            return xb

        def flush_pass2(pend):
            scores_sb, pmax_sb, xb, orow = pend
            # Transpose the RAW scores first (no softmax dependency), then
            # exponentiate in the [128, NC2] layout: the transposes never
            # wait on the softmax chain and the exp is 128-lane-parallel.
            sT = sm_pool.tile([128, NC2], F32, tag="sT")
            for g in range(NC2 // 8):
                tp = ps_t.tile([128, 8], F32, tag="tps")
                for u in range(8):
                    c2 = g * 8 + u
                    nc.tensor.matmul(
                        tp[:, u:u + 1], scores_sb[0:1, c2 * 128:(c2 + 1) * 128],
                        ident[0:1, 0:1],
                        is_transpose=True,
                        start=(u == 0), stop=(u == 7),
                        skip_group_check=True)
                nc.scalar.activation(sT[:, g * 8:(g + 1) * 8], tp[:], AF.Copy)

            # global max from the per-tile partials; broadcast -max to all
            # partitions with a K=1 matmul against a -1s row
            m_sb = sm_pool.tile([1, 1], F32, tag="m")
            nc.vector.reduce_max(m_sb[:], pmax_sb[:], axis=AX)
            mb_ps = ps_t.tile([128, 1], F32, tag="tps")
            nc.tensor.matmul(mb_ps[:], neg_ones[:], m_sb[:],
                             start=True, stop=True)
            mb = sm_pool.tile([128, 1], F32, tag="mb")
            nc.scalar.activation(mb[:], mb_ps[:], AF.Copy)

            # exp -> pass-2 stationary operand; accum gives per-partition Z
            pT = sm_pool.tile([128, NC2], F16, tag="pT")
            zc = sm_pool.tile([128, 1], F32, tag="zc")
            nc.scalar.activation(pT[:], sT[:], AF.Exp, bias=mb[:, 0:1],
                                 accum_out=zc[:])
            # cross-partition sum of zc -> Z -> 1/Z
            zt = ps_t.tile([1, 128], F32, tag="tps")
            nc.tensor.transpose(zt[:], zc[:], ident[:])
            zrow = sm_pool.tile([1, 128], F32, tag="zrow")
            nc.scalar.activation(zrow[:], zt[:], AF.Copy)
            z_sb = sm_pool.tile([1, 1], F32, tag="z")
            nc.vector.reduce_sum(z_sb[:], zrow[:], axis=AX)
            rz = sm_pool.tile([1, 1], F32, tag="rz")
            nc.vector.reciprocal(rz[:], z_sb[:])

            # pass 2: pooling
            op0 = ps_o.tile([1, 512], F32, tag="op0")
            op1 = ps_o.tile([1, 512], F32, tag="op1")
            for c2 in range(NC2):
                nc.tensor.matmul(op0[:], pT[:, c2:c2 + 1],
                                 xb[:, c2, 0:512],
                                 start=(c2 == 0), stop=(c2 == NC2 - 1))
                nc.tensor.matmul(op1[:], pT[:, c2:c2 + 1],
                                 xb[:, c2, 512:1024],
                                 start=(c2 == 0), stop=(c2 == NC2 - 1))

            ob = out_pool.tile([1, E], F32, tag="ob")
            nc.vector.tensor_scalar_mul(ob[:, 0:512], op0[:], rz[0:1, 0:1])
            nc.vector.tensor_scalar_mul(ob[:, 512:1024], op1[:], rz[0:1, 0:1])
            nc.sync.dma_start(o_d[orow:orow + 1, :], ob[:])

        # prologue: first xT transpose goes out before the W load so the
        # first matmul group can start ASAP
        xT_cur = dma_xt(tiles[0][1], tiles[0][2])
        for k in range(KE):
            nc.sync.dma_start(W_sb[:, k * A:(k + 1) * A],
                              W_d[k * 128:(k + 1) * 128, :])
        nc.sync.dma_start(b_sb[:], b_d.rearrange("(j p) -> p j", p=128))
        nc.sync.dma_start(ctx_sb[:], c_d.rearrange("(j p) -> p j", p=128))
        xT_next = dma_xt(tiles[1][1], tiles[1][2])

        # warm the PE HAM clock-gate with throwaway matmuls while the first
        # DMAs land (the PE would otherwise idle cold and re-throttle)
        warm_scratch = out_pool.tile([128, 512], F32, tag="ob",
                                     name="warm_scratch")
        for w in range(28):
            wp = ps_h.tile([128, 128], F32, tag="hps", name=f"warm{w}")
            nc.tensor.matmul(wp[:], ident[:], ident[:], start=True, stop=True)
            if w % 14 == 13:
                nc.scalar.activation(warm_scratch[:, 0:128], wp[:], AF.Copy)

        pending = None
        scores_sb = None
        ctx_q = []          # deferred ctx-dot matmuls (lag-1 behind tanh)

        for i, (rep, bi, t) in enumerate(tiles):
            if t == 0:
                scores_sb = sc_pool.tile([1, S], F32, tag="scores")
                pmax_sb = sc_pool.tile([1, NSUB], F32, tag="pmax")
                xb_cur = dma_xb(bi)

            if t == 1 and pending is not None:
                flush_pass2(pending)
                pending = None

            # prefetch the transposed tile two sub-tiles ahead
            if i + 2 < len(tiles):
                xT_pre = dma_xt(tiles[i + 2][1], tiles[i + 2][2])
            else:
                xT_pre = None

            sc_ps = ps_s.tile([1, ST], F32, tag="scps")
            for j in range(KA):
                hp = ps_h.tile([128, ST], F32, tag="hps")
                for k in range(KE):
                    nc.tensor.matmul(
                        hp[:],
                        W_sb[:, k * A + j * 128: k * A + (j + 1) * 128],
                        xT_cur[:, k, :],
                        start=(k == 0), stop=(k == KE - 1))
                # flush the previous ctx-dot now: its tanh ran during this
                # matmul group, so the PE never waits on the ScalarE chain
                while ctx_q:
                    ctx_q.pop(0)()
                h_sb = h_pool.tile([128, ST], F16, tag="h")
                nc.scalar.activation(h_sb[:], hp[:], AF.Tanh,
                                     bias=b_sb[:, j:j + 1])
                ctx_q.append(
                    lambda j=j, h_sb=h_sb, sc_ps=sc_ps: nc.tensor.matmul(
                        sc_ps[:], ctx_sb[:, j:j + 1], h_sb[:],
                        start=(j == 0), stop=(j == KA - 1)))
            # the j=7 ctx-dot drains at the start of the next sub-tile; the
            # score copy/max ride behind it in the deferred queue so they
            # are emitted only after the accumulation group is closed
            def copy_scores(scores_sb=scores_sb, pmax_sb=pmax_sb,
                            sc_ps=sc_ps, t=t):
                nc.vector.tensor_copy(
                    scores_sb[:, t * ST:(t + 1) * ST], sc_ps[:])
                nc.vector.reduce_max(pmax_sb[:, t:t + 1], sc_ps[:], axis=AX)
            ctx_q.append(copy_scores)
            if i == len(tiles) - 1:
                while ctx_q:
                    ctx_q.pop(0)()

            if t == NSUB - 1:
                pending = (scores_sb, pmax_sb, xb_cur, bi)

            xT_cur, xT_next = xT_next, xT_pre

        if pending is not None:
            flush_pass2(pending)

    nc.compile()
    return nc


_NC_CACHE = None


def kernel(x, W, b, ctx):
    global _NC_CACHE
    if _NC_CACHE is None:
        _NC_CACHE = _build()
    nc = _NC_CACHE

    x16 = np.ascontiguousarray(np.asarray(x).astype(np.float16))
    W16 = np.ascontiguousarray(np.asarray(W).astype(np.float16))
    b = np.ascontiguousarray(np.asarray(b, dtype=np.float32))
    c16 = np.ascontiguousarray(np.asarray(ctx).astype(np.float16))

    in_maps = [
        {"x": x16[i * BL:(i + 1) * BL].reshape(BL * S, E), "W": W16, "b": b,
         "ctx": c16}
        for i in range(NCORES)
    ]
    res = run_bass_kernel_spmd(nc, in_maps, core_ids=list(range(NCORES)))
    return np.concatenate([res.results[i]["out"] for i in range(NCORES)],
                          axis=0)


if __name__ == "__main__":
    rng = np.random.default_rng(0)
    x = rng.standard_normal((B, S, E), dtype=np.float32)
    W = rng.standard_normal((E, A), dtype=np.float32) / np.sqrt(E)
    b = rng.standard_normal((A,), dtype=np.float32) * 0.01
    c = rng.standard_normal((A,), dtype=np.float32)
    out = kernel(x=x, W=W, b=b, ctx=c)
    print(out.shape, out.dtype)


# revision 8
# speedup vs baseline: 1.1928x; 1.0500x over previous
"""Attention-pooling kernel for TRN2 (8 NeuronCores, batch-parallel).

Computes, for x:[32,2048,1024], W:[1024,1024], b:[1024], ctx:[1024]:
    h = tanh(x @ W + b); scores = h . ctx
    weights = softmax(scores, axis=seq)
    out = sum_s weights[s] * x[s]          -> [32, 1024]

Sharding: data-parallel over batch, 4 batches per core.

x and W are cast to fp16 on the host (10-bit mantissa keeps the score
error close to the f32r baseline) so the kernel can use the xbar DMA
transpose: each 512-row seq sub-tile of x is transposed HBM -> SBUF by
the DMA engines directly into a [128, 512, 8] tile with
tile[p, s, k] = x[s, 128k + p], i.e. each embed-block k is a stride-8
moving operand for the PE. This removes all PE transposes and their
PSUM-evacuation copies from the baseline. Pass 1 is then a pure fp16
matmul stream (full PE rate); tanh+bias on ScalarE; the ctx-dot rides
one accumulation group behind so the PE never waits on the tanh. The
softmax is unnormalized (exp(s - max); 1/Z lands on the pooled
vector). Pass 2 pools a straight fp16 copy of x with the exp'd weight
vector as the stationary operand, one batch behind pass 1.
"""

import numpy as np
from contextlib import ExitStack

import concourse.bacc as bacc
import concourse.mybir as mybir
import concourse.tile as tile
from concourse import masks
from concourse.bass_utils import run_bass_kernel_spmd

B, S, E, A = 32, 2048, 1024, 1024
NCORES = 8
BL = B // NCORES          # batches per core
ST = 512                  # seq sub-tile
NSUB = S // ST            # sub-tiles per batch
KE = E // 128             # contraction chunks over embed dim
KA = A // 128             # chunks over attention dim
NC2 = S // 128            # S chunks per batch (pass 2)

F32 = mybir.dt.float32
F16 = mybir.dt.float16
AX = mybir.AxisListType.X
AF = mybir.ActivationFunctionType


def _build(reps=1):
    nc = bacc.Bacc("TRN2", target_bir_lowering=False, debug=False,
                   num_devices=NCORES)
    x_d = nc.declare_dram_parameter("x", [BL * S, E], F16, isOutput=False)
    W_d = nc.declare_dram_parameter("W", [E, A], F16, isOutput=False)
    b_d = nc.declare_dram_parameter("b", [A], F32, isOutput=False)
    c_d = nc.declare_dram_parameter("ctx", [A], F16, isOutput=False)
    o_d = nc.declare_dram_parameter("out", [BL, E], F32, isOutput=True)

    with ExitStack() as ctx:
        tc = ctx.enter_context(tile.TileContext(nc))

        const_pool = ctx.enter_context(tc.tile_pool(name="const", bufs=1))
        xb_pool = ctx.enter_context(tc.tile_pool(name="xb", bufs=2))
        xt_pool = ctx.enter_context(tc.tile_pool(name="xT", bufs=3))
        h_pool = ctx.enter_context(tc.tile_pool(name="h", bufs=12))
        sc_pool = ctx.enter_context(tc.tile_pool(name="scores", bufs=2))
        sm_pool = ctx.enter_context(tc.tile_pool(name="softmax", bufs=1))
        out_pool = ctx.enter_context(tc.tile_pool(name="outs", bufs=1))

        ps_h = ctx.enter_context(tc.tile_pool(name="ps_h", bufs=2, space="PSUM"))
        ps_s = ctx.enter_context(tc.tile_pool(name="ps_s", bufs=2, space="PSUM"))
        ps_t = ctx.enter_context(tc.tile_pool(name="ps_t", bufs=2, space="PSUM"))
        ps_o = ctx.enter_context(tc.tile_pool(name="ps_o", bufs=1, space="PSUM"))

        # ---- constants ----
        ident = const_pool.tile([128, 128], F32)
        masks.make_identity(nc, ident[:])
        neg_ones = const_pool.tile([1, 128], F32)
        nc.gpsimd.memset(neg_ones[:], -1.0)

        W_sb = const_pool.tile([128, KE * A], F16)
        b_sb = const_pool.tile([128, KA], F32)
        ctx_sb = const_pool.tile([128, KA], F16)

        tiles = [(rep, bi, t)
                 for rep in range(reps) for bi in range(BL) for t in range(NSUB)]

        def dma_xt(bi, t):
            # xbar DMA transpose: x rows [512, 1024] -> [128, 8, 512] with
            # xT[p, k, s] = x[r0 + s, 128k + p] (the k dim is "logically part
            # of the partition dim": out logical column e = 128k + p)
            r0 = bi * S + t * ST
            xT = xt_pool.tile([128, KE, ST], F16, tag="xT")
            nc.sync.dma_start_transpose(xT[:], x_d[r0:r0 + ST, :])
            return xT

        def dma_xb(bi):
            # straight fp16 copy for pass-2 pooling: [128, 16, 1024] with
            # xb[p, c, e] = x[bi*S + 128c + p, e]
            xb = xb_pool.tile([128, NC2, E], F16, tag="xb")
            nc.sync.dma_start(
                xb[:], x_d[bi * S:(bi + 1) * S, :].rearrange(
                    "(c p) e -> p c e", p=128))
            return xb

        def flush_pass2(pend):
            scores_sb, pmax_sb, xb, orow = pend
            # Transpose the RAW scores first (no softmax dependency), then
            # exponentiate in the [128, NC2] layout: the transposes never
            # wait on the softmax chain and the exp is 128-lane-parallel.
            sT = sm_pool.tile([128, NC2], F32, tag="sT")
            for g in range(NC2 // 8):
                tp = ps_t.tile([128, 8], F32, tag="tps")
                for u in range(8):
                    c2 = g * 8 + u
                    nc.tensor.matmul(
                        tp[:, u:u + 1], scores_sb[0:1, c2 * 128:(c2 + 1) * 128],
                        ident[0:1, 0:1],
                        is_transpose=True,
                        start=(u == 0), stop=(u == 7),
                        skip_group_check=True)
                nc.scalar.activation(sT[:, g * 8:(g + 1) * 8], tp[:], AF.Copy)

            # global max from the per-tile partials; broadcast -max to all
            # partitions with a K=1 matmul against a -1s row
            m_sb = sm_pool.tile([1, 1], F32, tag="m")
            nc.vector.reduce_max(m_sb[:], pmax_sb[:], axis=AX)
            mb_ps = ps_t.tile([128, 1], F32, tag="tps")
            nc.tensor.matmul(mb_ps[:], neg_ones[:], m_sb[:],
                             start=True, stop=True)
            mb = sm_pool.tile([128, 1], F32, tag="mb")
            nc.scalar.activation(mb[:], mb_ps[:], AF.Copy)

            # exp -> pass-2 stationary operand; accum gives per-partition Z
            pT = sm_pool.tile([128, NC2], F16, tag="pT")
            zc = sm_pool.tile([128, 1], F32, tag="zc")
            nc.scalar.activation(pT[:], sT[:], AF.Exp, bias=mb[:, 0:1],
                                 accum_out=zc[:])
            # cross-partition sum of zc -> Z -> 1/Z
            zt = ps_t.tile([1, 128], F32, tag="tps")
            nc.tensor.transpose(zt[:], zc[:], ident[:])
            zrow = sm_pool.tile([1, 128], F32, tag="zrow")
            nc.scalar.activation(zrow[:], zt[:], AF.Copy)
            z_sb = sm_pool.tile([1, 1], F32, tag="z")
            nc.vector.reduce_sum(z_sb[:], zrow[:], axis=AX)
            rz = sm_pool.tile([1, 1], F32, tag="rz")
            nc.vector.reciprocal(rz[:], z_sb[:])

            # pass 2: pooling
            op0 = ps_o.tile([1, 512], F32, tag="op0")
            op1 = ps_o.tile([1, 512], F32, tag="op1")
            # two uninterleaved accumulation groups: alternating the PSUM
            # target bank every matmul costs ~2x95ns per switch on HW
            for c2 in range(NC2):
                nc.tensor.matmul(op0[:], pT[:, c2:c2 + 1],
                                 xb[:, c2, 0:512],
                                 start=(c2 == 0), stop=(c2 == NC2 - 1))
            for c2 in range(NC2):
                nc.tensor.matmul(op1[:], pT[:, c2:c2 + 1],
                                 xb[:, c2, 512:1024],
                                 start=(c2 == 0), stop=(c2 == NC2 - 1))

            ob = out_pool.tile([1, E], F32, tag="ob")
            nc.vector.tensor_scalar_mul(ob[:, 0:512], op0[:], rz[0:1, 0:1])
            nc.vector.tensor_scalar_mul(ob[:, 512:1024], op1[:], rz[0:1, 0:1])
            nc.sync.dma_start(o_d[orow:orow + 1, :], ob[:])

        # prologue: first xT transpose goes out before the W load so the
        # first matmul group can start ASAP
        xT_cur = dma_xt(tiles[0][1], tiles[0][2])
        for k in range(KE):
            nc.sync.dma_start(W_sb[:, k * A:(k + 1) * A],
                              W_d[k * 128:(k + 1) * 128, :])
        nc.sync.dma_start(b_sb[:], b_d.rearrange("(j p) -> p j", p=128))
        nc.sync.dma_start(ctx_sb[:], c_d.rearrange("(j p) -> p j", p=128))
        xT_next = dma_xt(tiles[1][1], tiles[1][2])

        # warm the PE HAM clock-gate with throwaway matmuls while the first
        # DMAs land (the PE would otherwise idle cold and re-throttle)
        warm_scratch = out_pool.tile([128, 512], F32, tag="ob",
                                     name="warm_scratch")
        for w in range(28):
            wp = ps_h.tile([128, 128], F32, tag="hps", name=f"warm{w}")
            nc.tensor.matmul(wp[:], ident[:], ident[:], start=True, stop=True)
            if w % 14 == 13:
                nc.scalar.activation(warm_scratch[:, 0:128], wp[:], AF.Copy)

        pending = None
        scores_sb = None
        ctx_q = []          # deferred ctx-dot matmuls (lag-1 behind tanh)

        for i, (rep, bi, t) in enumerate(tiles):
            if t == 0:
                scores_sb = sc_pool.tile([1, S], F32, tag="scores")
                pmax_sb = sc_pool.tile([1, NSUB], F32, tag="pmax")
                xb_cur = dma_xb(bi)

            if t == 1 and pending is not None:
                flush_pass2(pending)
                pending = None

            # prefetch the transposed tile two sub-tiles ahead
            if i + 2 < len(tiles):
                xT_pre = dma_xt(tiles[i + 2][1], tiles[i + 2][2])
            else:
                xT_pre = None

            sc_ps = ps_s.tile([1, ST], F32, tag="scps")
            for j in range(KA):
                hp = ps_h.tile([128, ST], F32, tag="hps")
                for k in range(KE):
                    nc.tensor.matmul(
                        hp[:],
                        W_sb[:, k * A + j * 128: k * A + (j + 1) * 128],
                        xT_cur[:, k, :],
                        start=(k == 0), stop=(k == KE - 1))
                # drain the whole previous sub-tile's ctx-dot as ONE
                # contiguous group (all its tanhs are long done): entering/
                # leaving a ctx matmul mid-stream costs ~2x95ns on HW, so
                # batching 8 of them pays ~16x less switch penalty
                if j == 1:
                    while ctx_q:
                        ctx_q.pop(0)()
                h_sb = h_pool.tile([128, ST], F16, tag="h")
                nc.scalar.activation(h_sb[:], hp[:], AF.Tanh,
                                     bias=b_sb[:, j:j + 1])
                ctx_q.append(
                    lambda j=j, h_sb=h_sb, sc_ps=sc_ps: nc.tensor.matmul(
                        sc_ps[:], ctx_sb[:, j:j + 1], h_sb[:],
                        start=(j == 0), stop=(j == KA - 1)))
            # the j=7 ctx-dot drains at the start of the next sub-tile; the
            # score copy/max ride behind it in the deferred queue so they
            # are emitted only after the accumulation group is closed
            def copy_scores(scores_sb=scores_sb, pmax_sb=pmax_sb,
                            sc_ps=sc_ps, t=t):
                nc.vector.tensor_copy(
                    scores_sb[:, t * ST:(t + 1) * ST], sc_ps[:])
                nc.vector.reduce_max(pmax_sb[:, t:t + 1], sc_ps[:], axis=AX)
            ctx_q.append(copy_scores)
            if i == len(tiles) - 1:
                while ctx_q:
                    ctx_q.pop(0)()

            if t == NSUB - 1:
                pending = (scores_sb, pmax_sb, xb_cur, bi)

            xT_cur, xT_next = xT_next, xT_pre

        if pending is not None:
            flush_pass2(pending)

    nc.compile()
    return nc


_NC_CACHE = None


def kernel(x, W, b, ctx):
    global _NC_CACHE
    if _NC_CACHE is None:
        _NC_CACHE = _build()
    nc = _NC_CACHE

    x16 = np.ascontiguousarray(np.asarray(x).astype(np.float16))
    W16 = np.ascontiguousarray(np.asarray(W).astype(np.float16))
    b = np.ascontiguousarray(np.asarray(b, dtype=np.float32))
    c16 = np.ascontiguousarray(np.asarray(ctx).astype(np.float16))

    in_maps = [
        {"x": x16[i * BL:(i + 1) * BL].reshape(BL * S, E), "W": W16, "b": b,
         "ctx": c16}
        for i in range(NCORES)
    ]
    res = run_bass_kernel_spmd(nc, in_maps, core_ids=list(range(NCORES)))
    return np.concatenate([res.results[i]["out"] for i in range(NCORES)],
                          axis=0)


if __name__ == "__main__":
    rng = np.random.default_rng(0)
    x = rng.standard_normal((B, S, E), dtype=np.float32)
    W = rng.standard_normal((E, A), dtype=np.float32) / np.sqrt(E)
    b = rng.standard_normal((A,), dtype=np.float32) * 0.01
    c = rng.standard_normal((A,), dtype=np.float32)
    out = kernel(x=x, W=W, b=b, ctx=c)
    print(out.shape, out.dtype)
